# revision 17
# baseline (speedup 1.0000x reference)
"""Trainium2 Bass kernel for nn_ChildHAggregation (gnn_message_passing).

Computation per sample (see docstring math below):
  x = [hl, hr]                                        (1024)
  q_t = (h_t @ qU + qU_b) * su_q + sb_q   t in {l,r}  su/sb from xh
  k_t = (h_t @ kU + kU_b) * su_k + sb_k
  2x2 softmax attention over the two tokens -> per-sample probs p_ij
  x += scores @ [hl, hr]; layernorm(ddof=1) with alpha/beta
  out = (x @ hU + hU_b) * (xh @ hWu + hWu_b) + (xh @ hWb + hWb_b)
      + (xw @ lU + lU_b) * (xh @ lWu + lWu_b) + (xh @ lWb + lWb_b)

Strategy (pure data-parallel over 8 cores, batch 32768 -> 4096/core):
  - batch-major tiles [128 samples x features]; feature-contraction
    matmuls use PE-transposed input tiles as the stationary operand so
    outputs land batch-major in PSUM.
  - 2-token softmax == sigmoid of score differences -> per-sample
    scalars; attention + layernorm are folded into per-sample scalar
    algebra applied AFTER the matmuls:
      x @ (alpha*hU) = a0*M1 + b0*M2 + a1*M3 + b1*M4  (M* = h_t @ hU'half)
      norm fixup: inv * (hu_x - mean * colsum(alpha*hU)), beta@hU folded
      into a bias row.
  - all "U-bias * hyper" terms folded into precomputed weights:
      qWb' = qWb + qWu*diag(qU_b) etc., so q = (h@qU)*su + (xh@qWb'+qb').
    hidden/leaf additive paths combined: WC = hWb' + lWb' (one matmul).
"""

import os
from contextlib import ExitStack

import numpy as np

import concourse.bacc as bacc
import concourse.bass as bass
import concourse.mybir as mybir
import concourse.tile as tile
from concourse.bass_utils import run_bass_kernel_spmd
from concourse.masks import make_identity

N_CORES = 8
B_FULL = 32768
HALF = 512
DIM = 1024
P = 128
EPS = 1e-6
INV_SQRT_HALF = 1.0 / float(np.sqrt(np.float32(HALF)))

f32 = mybir.dt.float32
f32r = mybir.dt.float32r

AX = mybir.AxisListType
ALU = mybir.AluOpType
ACTF = mybir.ActivationFunctionType

W512 = ["qU", "kU", "qWu", "qWb", "kWu", "kWb", "hWu", "hWb", "lWu", "lWb"]


def _mm(ap, mm_dt):
    """Bitcast an fp32 AP to the matmul dtype (f32 or f32r; same bits)."""
    if mm_dt == f32:
        return ap
    return ap.bitcast(mm_dt)


def build_nc(b_loc, mm_dt=f32r, stage=99):
    """Build the per-core Bass program for a local batch of b_loc rows.

    stage truncates the program for hang-bisection:
      1=setup only, 2=+transposes, 3=+phaseA mm/evict, 4=+qk/dots/probs,
      5=+stats algebra, 99=full
    """
    n_tiles = b_loc // P
    assert n_tiles * P == b_loc

    nc = bacc.Bacc("TRN2", target_bir_lowering=False, debug=False,
                   num_devices=1)

    # ---- DRAM I/O (names match setup_inputs) ----
    d = {}
    d["hl"] = nc.dram_tensor("hl", [b_loc, HALF], f32, kind="ExternalInput").ap()
    d["hr"] = nc.dram_tensor("hr", [b_loc, HALF], f32, kind="ExternalInput").ap()
    d["xw"] = nc.dram_tensor("xw", [b_loc, DIM], f32, kind="ExternalInput").ap()
    d["xh"] = nc.dram_tensor("xh", [b_loc, HALF], f32, kind="ExternalInput").ap()
    for w in W512:
        d[w + "_w"] = nc.dram_tensor(w + "_w", [HALF, HALF], f32,
                                     kind="ExternalInput").ap()
        d[w + "_b"] = nc.dram_tensor(w + "_b", [HALF], f32,
                                     kind="ExternalInput").ap()
    for w in ["hU", "lU"]:
        d[w + "_w"] = nc.dram_tensor(w + "_w", [DIM, HALF], f32,
                                     kind="ExternalInput").ap()
        d[w + "_b"] = nc.dram_tensor(w + "_b", [HALF], f32,
                                     kind="ExternalInput").ap()
    d["alpha"] = nc.dram_tensor("alpha", [DIM], f32, kind="ExternalInput").ap()
    d["beta"] = nc.dram_tensor("beta", [DIM], f32, kind="ExternalInput").ap()
    out_d = nc.dram_tensor("out", [b_loc, HALF], f32, kind="ExternalOutput").ap()

    with tile.TileContext(nc) as tc, ExitStack() as ctx:
        # ================= persistent pools =================
        wts = ctx.enter_context(tc.tile_pool(name="wts", bufs=1))
        biasp = ctx.enter_context(tc.tile_pool(name="biasp", bufs=1))
        # PSUM pools opened up-front (8 banks total); setup matmuls borrow
        # "mm" tiles so no extra pool serializes against the main loop.
        tp_ps = ctx.enter_context(tc.tile_pool(name="tp_ps", bufs=2, space="PSUM"))
        mm_ps = ctx.enter_context(tc.tile_pool(name="mm_ps", bufs=6, space="PSUM"))

        # final weight tiles (written exactly once, in the matmul dtype, so
        # the fp32r BIR-verifier sees rounded producers)
        wsb = {}
        for w in ["qU", "kU", "qWu", "kWu", "hWu", "lWu", "qWb", "hWb"]:
            wsb[w] = wts.tile([P, 4, HALF], f32, name=f"w_{w}")
        for w in ["hU", "lU"]:
            wsb[w] = wts.tile([P, 8, HALF], f32, name=f"w_{w}")
        alpha_sb = wts.tile([P, 8], f32)
        nc.sync.dma_start(alpha_sb, d["alpha"].rearrange("(c p) -> p c", p=P))
        beta_sb = wts.tile([P, 8], f32)
        nc.sync.dma_start(beta_sb, d["beta"].rearrange("(c p) -> p c", p=P))
        ident = wts.tile([P, P], f32)
        make_identity(nc, ident)

        # persistent broadcast [P, 512] bias tiles
        bc = {}
        for nm in ["qWu_b", "kWu_b", "hWu_b", "lWu_b", "qb", "cb", "cs"]:
            bc[nm] = biasp.tile([P, HALF], f32, name=f"bc_{nm}")

        # ================= input pool + prefetch =================
        inp = ctx.enter_context(tc.tile_pool(name="inp", bufs=2))
        tsp = ctx.enter_context(tc.tile_pool(name="tsp", bufs=2))
        prefetched = {}
        for i in range(min(2, n_tiles)):
            for nm, wdt in (("hl", HALF), ("hr", HALF), ("xh", HALF),
                            ("xw", DIM)):
                t = inp.tile([P, wdt], f32, tag=nm, name=f"pre_{nm}_{i}")
                nc.sync.dma_start(t, d[nm][bass.ts(i, P), :])
                prefetched[(i, nm)] = t

        # ---- transpose helper (used pre-loop and in-loop) ----
        def transpose_to(src, ncols, tg, i):
            sb = tsp.tile([P, ncols * P], f32, tag=tg, name=f"T_{tg}_{i}")
            for g in range(0, ncols, 4):
                ps = tp_ps.tile([P, 4 * P], f32, tag="tp",
                                name=f"tps_{tg}_{g}_{i}")
                gw = min(4, ncols - g)
                for c in range(gw):
                    nc.tensor.transpose(
                        ps[:, c * P:(c + 1) * P],
                        src[:, (g + c) * P:(g + c + 1) * P],
                        ident)
                # evict writes the matmul dtype (rounds for fp32r)
                nc.scalar.copy(_mm(sb[:, g * P:(g + gw) * P], mm_dt),
                               ps[:, :gw * P])
            return sb

        # ---- pre-loop transposes for tiles 0-1: keeps PE busy from t~=6us
        # while weight DMAs land (they only need inputs + identity) ----
        preT = {}
        prehd = {}
        for i in range(min(2, n_tiles)):
            hd_t = inp.tile([P, HALF], f32, tag="hd", name=f"pre_hd_{i}")
            nc.vector.tensor_sub(hd_t, prefetched[(i, "hl")],
                                 prefetched[(i, "hr")])
            prehd[i] = hd_t
            preT[(i, "ThL")] = transpose_to(prefetched[(i, "hl")], 4, "ThL", i)
            preT[(i, "ThR")] = transpose_to(prefetched[(i, "hr")], 4, "ThR", i)
            preT[(i, "ThD")] = transpose_to(hd_t, 4, "ThD", i)
            preT[(i, "TxH")] = transpose_to(prefetched[(i, "xh")], 4, "TxH", i)
            preT[(i, "TxW")] = transpose_to(prefetched[(i, "xw")], 8, "TxW", i)

        # ---------------- one-time setup ----------------
        with tc.tile_pool(name="setup", bufs=1) as sp:

            _rows = {}

            def bias_row(nm, dedicated=False):
                if nm not in _rows:
                    tag = f"row_{nm}" if dedicated else "row"
                    r = sp.tile([1, HALF], f32, tag=tag,
                                bufs=(1 if dedicated else 2), name=f"row_{nm}")
                    nc.sync.dma_start(r, d[nm][None, :])
                    _rows[nm] = r
                return _rows[nm]

            # late-consumed rows get dedicated buffers + their DMAs issued
            # up-front so they don't queue behind the multi-MB weight loads
            for nm in ["hU_b", "hWb_b", "lWb_b"]:
                bias_row(nm, dedicated=True)

            def bcast(dst, row_ap):
                nc.gpsimd.partition_broadcast(dst, row_ap)

            def tmp_bc(nm, row_ap):
                t = sp.tile([P, HALF], f32, tag="tbc", bufs=4, name=f"tbc_{nm}")
                bcast(t, row_ap)
                return t

            # su-evict biases straight to persistent broadcasts
            for nm in ["qWu_b", "kWu_b", "hWu_b", "lWu_b"]:
                bcast(bc[nm], bias_row(nm))

            # temp broadcasts used by folds / combined rows
            qUb_bc = tmp_bc("qU_b", bias_row("qU_b"))
            lUb_bc = tmp_bc("lU_b", bias_row("lU_b"))

            # combined bias rows (computed on broadcast tiles):
            # qb' = qWb_b + qU_b*qWu_b ; kb' likewise
            ftmp = sp.tile([P, HALF], f32, tag="ftmp")
            bcast(bc["qb"], bias_row("qWb_b"))
            nc.vector.tensor_mul(ftmp, qUb_bc, bc["qWu_b"])
            nc.vector.tensor_add(bc["qb"], bc["qb"], ftmp)

            # weights: DMA into rotating temps (one DMA per 128-row chunk
            # so transfers spread across DMA queues), finals written once.
            # All temps are 4-chunk [P,4,HALF] tiles on one rotating tag;
            # 8-chunk weights (hU/lU) load as two halves.
            def wtemp(w, c0=0, nch=4):
                t = sp.tile([P, nch, HALF], f32, tag="wtmp4", bufs=3,
                            name=f"wtmp_{w}_{c0}")
                rr = d[w + "_w"].rearrange("(c p) o -> p c o", p=P)
                for c in range(nch):
                    nc.sync.dma_start(t[:, c, :], rr[:, c0 + c, :])
                return t

            # hU first: the setup matmuls (beta@hU / alpha@hU) are at the
            # head of the PE queue after the pre-loop transposes, so its DMA
            # is the critical path for PE to keep running.
            hU_tmps = [wtemp("hU", 0), wtemp("hU", 4)]
            bh_full = mm_ps.tile([P, HALF], f32, tag="mm", name="ps_bh_setup")
            cs_full = mm_ps.tile([P, HALF], f32, tag="mm", name="ps_cs_setup")
            bh_ps = bh_full[0:1, :]
            cs_ps = cs_full[0:1, :]
            for c in range(8):
                nc.tensor.matmul(bh_ps, beta_sb[:, c:c + 1],
                                 hU_tmps[c // 4][:, c % 4, :],
                                 start=(c == 0), stop=(c == 7))
            for c in range(8):
                nc.tensor.matmul(cs_ps, alpha_sb[:, c:c + 1],
                                 hU_tmps[c // 4][:, c % 4, :],
                                 start=(c == 0), stop=(c == 7))
            bh_row = sp.tile([1, HALF], f32, tag="row", bufs=2)
            nc.vector.tensor_add(bh_row, bh_ps, bias_row("hU_b"))
            cs_row = sp.tile([1, HALF], f32, tag="row", bufs=2)
            nc.vector.tensor_copy(cs_row, cs_ps)
            bcast(bc["cs"], cs_row)
            bh_bc = tmp_bc("bh", bh_row)
            for c in range(8):
                nc.vector.tensor_scalar_mul(_mm(wsb["hU"][:, c, :], mm_dt),
                                            hU_tmps[c // 4][:, c % 4, :],
                                            alpha_sb[:, c:c + 1])

            wt_tmps = {}
            for w in ["qU", "kU", "qWu", "kWu"]:
                t = wt_tmps[w] = wtemp(w)
                for c in range(4):
                    nc.vector.tensor_copy(_mm(wsb[w][:, c, :], mm_dt), t[:, c, :])
            for w in ["hWu", "lWu"]:
                t = wt_tmps[w] = wtemp(w)
                for c in range(4):
                    nc.vector.tensor_copy(_mm(wsb[w][:, c, :], mm_dt), t[:, c, :])

            for half in range(2):
                lU_tmp = wtemp("lU", 4 * half)
                for c in range(4):
                    nc.vector.tensor_copy(
                        _mm(wsb["lU"][:, 4 * half + c, :], mm_dt),
                        lU_tmp[:, c, :])

            # cb = (hWb_b + bh*hWu_b) + (lWb_b + lU_b*lWu_b)
            bcast(bc["cb"], bias_row("hWb_b"))
            nc.vector.tensor_mul(ftmp, bh_bc, bc["hWu_b"])
            nc.vector.tensor_add(bc["cb"], bc["cb"], ftmp)
            lWbb_bc = tmp_bc("lWb_b", bias_row("lWb_b"))
            nc.vector.tensor_add(bc["cb"], bc["cb"], lWbb_bc)
            nc.vector.tensor_mul(ftmp, lUb_bc, bc["lWu_b"])
            nc.vector.tensor_add(bc["cb"], bc["cb"], ftmp)

            # folded weight matrices:
            # qWb' = qWb + qWu*diag(qU_b) ; kWb' = kWb + kWu*diag(kU_b)
            # WC   = hWb + hWu*diag(bh) + lWb + lWu*diag(lU_b)
            qWb_tmp = wtemp("qWb")
            for c in range(4):
                nc.vector.tensor_mul(ftmp, wsb["qWu"][:, c, :], qUb_bc)
                nc.vector.tensor_add(_mm(wsb["qWb"][:, c, :], mm_dt),
                                     qWb_tmp[:, c, :], ftmp)
            hWb_tmp = wtemp("hWb")
            lWb_tmp = wtemp("lWb")
            for c in range(4):
                nc.vector.tensor_mul(ftmp, wsb["hWu"][:, c, :], bh_bc)
                nc.vector.tensor_add(hWb_tmp[:, c, :], hWb_tmp[:, c, :], ftmp)
                nc.vector.tensor_add(hWb_tmp[:, c, :], hWb_tmp[:, c, :],
                                     lWb_tmp[:, c, :])
                nc.vector.tensor_mul(ftmp, wsb["lWu"][:, c, :], lUb_bc)
                nc.vector.tensor_add(_mm(wsb["hWb"][:, c, :], mm_dt),
                                     hWb_tmp[:, c, :], ftmp)

        # ================= main loop pools =================
        pha = ctx.enter_context(tc.tile_pool(name="pha", bufs=1))
        scr = ctx.enter_context(tc.tile_pool(name="scr", bufs=3))
        tinyp = ctx.enter_context(tc.tile_pool(name="tinyp", bufs=2))
        phd = ctx.enter_context(tc.tile_pool(name="phd", bufs=1))
        outp = ctx.enter_context(tc.tile_pool(name="outp", bufs=2))

        for i in range(n_tiles):
            rs = bass.ts(i, P)
            # ---- loads ----
            if (i, "hl") in prefetched:
                hl_t = prefetched.pop((i, "hl"))
                hr_t = prefetched.pop((i, "hr"))
                xh_t = prefetched.pop((i, "xh"))
                xw_t = prefetched.pop((i, "xw"))
                hd_t = prehd.pop(i)
            else:
                hl_t = inp.tile([P, HALF], f32, tag="hl")
                nc.sync.dma_start(hl_t, d["hl"][rs, :])
                hr_t = inp.tile([P, HALF], f32, tag="hr")
                nc.sync.dma_start(hr_t, d["hr"][rs, :])
                xh_t = inp.tile([P, HALF], f32, tag="xh")
                nc.sync.dma_start(xh_t, d["xh"][rs, :])
                xw_t = inp.tile([P, DIM], f32, tag="xw")
                nc.sync.dma_start(xw_t, d["xw"][rs, :])
                # hd = hl - hr (single-unit kU matmul on the difference)
                hd_t = inp.tile([P, HALF], f32, tag="hd")
                nc.vector.tensor_sub(hd_t, hl_t, hr_t)


            def stage_out(src_ap):
                ot = outp.tile([P, HALF], f32, tag="out_t", name=f"out_stage_{i}")
                nc.vector.tensor_copy(ot, src_ap)
                nc.sync.dma_start(out_d[rs, :], ot)

            if stage == 1:
                stage_out(hl_t)
                continue

            # ---- PE transposes (feature-major stationaries) ----
            if (i, "ThL") in preT:
                hlT = preT.pop((i, "ThL"))
                hrT = preT.pop((i, "ThR"))
                hdT = preT.pop((i, "ThD"))
                xhT = preT.pop((i, "TxH"))
                xwT = preT.pop((i, "TxW"))
            else:
                hlT = transpose_to(hl_t, 4, "ThL", i)
                hrT = transpose_to(hr_t, 4, "ThR", i)
                hdT = transpose_to(hd_t, 4, "ThD", i)
                xhT = transpose_to(xh_t, 4, "TxH", i)
                xwT = transpose_to(xw_t, 8, "TxW", i)

            if stage == 2:
                stage_out(hlT)
                continue

            # ---- row stats of hl / hr (for fused layernorm algebra) ----
            skip_stats = (stage == 22)
            smask = int(os.environ.get("KERNEL_STATS_MASK", "7"))
            if not skip_stats:
                sl = tinyp.tile([P, 1], f32, tag="sl")
                sr = tinyp.tile([P, 1], f32, tag="sr")
                ql = tinyp.tile([P, 1], f32, tag="ql")
                qr = tinyp.tile([P, 1], f32, tag="qr")
                cr2 = tinyp.tile([P, 1], f32, tag="cr2")
                if smask & 1:
                    s4 = scr.tile([P, HALF], f32, tag="scr", name=f"scr_sl_{i}")
                    nc.scalar.activation(s4, hl_t, ACTF.Copy, accum_out=sl)
                    s5 = scr.tile([P, HALF], f32, tag="scr", name=f"scr_sr_{i}")
                    nc.scalar.activation(s5, hr_t, ACTF.Copy, accum_out=sr)
                if smask & 2:
                    s1 = scr.tile([P, HALF], f32, tag="scr", name=f"scr_ql_{i}")
                    nc.vector.scalar_tensor_tensor(
                        s1, hl_t, 0.0, hl_t, ALU.bypass, ALU.mult, accum_out=ql)
                    s2 = scr.tile([P, HALF], f32, tag="scr", name=f"scr_qr_{i}")
                    nc.vector.scalar_tensor_tensor(
                        s2, hr_t, 0.0, hr_t, ALU.bypass, ALU.mult, accum_out=qr)
                if smask & 4:
                    # 2*hl*hr: the factor 2 folds the old f2 doubling
                    s3 = scr.tile([P, HALF], f32, tag="scr", name=f"scr_cr_{i}")
                    nc.vector.scalar_tensor_tensor(
                        s3, hl_t, 2.0, hr_t, ALU.mult, ALU.mult,
                        accum_out=cr2)
            if stage == 21:
                stage_out(hlT)
                continue

            # ---- phase A matmuls ----
            def unit(tag):
                return mm_ps.tile([P, HALF], f32, tag="mm", name=f"ps_{tag}_{i}")

            SUq, SBq, TU = unit("SUq"), unit("SBq"), unit("TU")
            for c in range(4):
                lhs = _mm(xhT[:, bass.ts(c, P)], mm_dt)
                st, sp_ = (c == 0), (c == 3)
                nc.tensor.matmul(SUq, lhs, _mm(wsb["qWu"][:, c, :], mm_dt), start=st, stop=sp_)
                nc.tensor.matmul(SBq, lhs, _mm(wsb["qWb"][:, c, :], mm_dt), start=st, stop=sp_)
                nc.tensor.matmul(TU, lhs, _mm(wsb["kWu"][:, c, :], mm_dt), start=st, stop=sp_)
            A_l = unit("A_l")
            for c in range(4):
                nc.tensor.matmul(A_l, _mm(hlT[:, bass.ts(c, P)], mm_dt),
                                 _mm(wsb["qU"][:, c, :], mm_dt),
                                 start=(c == 0), stop=(c == 3))
            A_r = unit("A_r")
            for c in range(4):
                nc.tensor.matmul(A_r, _mm(hrT[:, bass.ts(c, P)], mm_dt),
                                 _mm(wsb["qU"][:, c, :], mm_dt),
                                 start=(c == 0), stop=(c == 3))
            CD = unit("CD")
            for c in range(4):
                nc.tensor.matmul(CD, _mm(hdT[:, bass.ts(c, P)], mm_dt),
                                 _mm(wsb["kU"][:, c, :], mm_dt),
                                 start=(c == 0), stop=(c == 3))

            # ---- phase A elementwise (score-difference trick) ----
            # d0 = q_l . (k_l - k_r), d1 = q_r . (k_l - k_r);
            # k_l - k_r = (C_l - C_r) * tu  (kU_b and the k additive
            # hyper term cancel in the difference), and with the qWb'
            # fold q_t = A_t * su + sbq, so
            # d_t = sum(A_t * su * dk) + sum(sbq * dk).
            su = pha.tile([P, HALF], f32, tag="su", bufs=2)
            nc.vector.tensor_add(su, SUq, bc["qWu_b"])
            sbq = pha.tile([P, HALF], f32, tag="sbq", bufs=2)
            nc.vector.tensor_add(sbq, SBq, bc["qb"])
            tu = pha.tile([P, HALF], f32, tag="tu", bufs=2)
            nc.vector.tensor_add(tu, TU, bc["kWu_b"])

            if stage == 3:
                stage_out(su)
                continue

            dk = pha.tile([P, HALF], f32, tag="dk")
            nc.vector.tensor_mul(dk, CD, tu)
            u = pha.tile([P, HALF], f32, tag="u")
            nc.vector.tensor_mul(u, su, dk)

            stats = tinyp.tile([P, 4], f32, tag="stats")
            for j, (aa, bb) in enumerate([(sbq, dk), (A_l, u), (A_r, u)]):
                sd = scr.tile([P, HALF], f32, tag="scr", name=f"scr_dot{j}_{i}")
                nc.vector.scalar_tensor_tensor(
                    sd, aa, 0.0, bb, ALU.bypass, ALU.mult,
                    accum_out=stats[:, j:j + 1])

            # ---- 2-way softmax via sigmoid ----
            diffs = tinyp.tile([P, 2], f32, tag="diffs")
            nc.vector.tensor_add(diffs, stats[:, 1:3],
                                 stats[:, 0:1].broadcast_to([P, 2]))
            probs = tinyp.tile([P, 2], f32, tag="probs")
            nc.scalar.activation(probs, diffs, ACTF.Sigmoid, scale=INV_SQRT_HALF)
            # a/b combine coefficients on gpsimd (keeps the scalar queue free
            # for transpose evictions)
            a0 = tinyp.tile([P, 1], f32, tag="a0")
            nc.gpsimd.tensor_scalar_add(a0, probs[:, 0:1], 1.0)
            b0 = tinyp.tile([P, 1], f32, tag="b0")
            nc.gpsimd.tensor_scalar(b0, probs[:, 0:1], -1.0, 1.0,
                                    ALU.mult, ALU.add)
            a1 = probs[:, 1:2]
            b1 = tinyp.tile([P, 1], f32, tag="b1")
            nc.gpsimd.tensor_scalar(b1, probs[:, 1:2], -1.0, 2.0,
                                    ALU.mult, ALU.add)

            if stage == 4:
                stage_out(u)
                continue

            # ---- layernorm stats from folded algebra ----
            e0 = tinyp.tile([P, 1], f32, tag="e0")
            nc.vector.tensor_add(e0, a0, a1)
            e1 = tinyp.tile([P, 1], f32, tag="e1")
            nc.vector.tensor_add(e1, b0, b1)
            sumx = tinyp.tile([P, 1], f32, tag="sumx")
            nc.vector.tensor_mul(sumx, sl, e0)
            nc.vector.scalar_tensor_tensor(sumx, sr, e1, sumx, ALU.mult, ALU.add)
            f0 = tinyp.tile([P, 1], f32, tag="f0")
            nc.vector.tensor_mul(f0, a0, a0)
            nc.vector.scalar_tensor_tensor(f0, a1, a1, f0, ALU.mult, ALU.add)
            f1 = tinyp.tile([P, 1], f32, tag="f1")
            nc.vector.tensor_mul(f1, b0, b0)
            nc.vector.scalar_tensor_tensor(f1, b1, b1, f1, ALU.mult, ALU.add)
            f2 = tinyp.tile([P, 1], f32, tag="f2")
            nc.vector.tensor_mul(f2, a0, b0)
            nc.vector.scalar_tensor_tensor(f2, a1, b1, f2, ALU.mult, ALU.add)
            ssq = tinyp.tile([P, 1], f32, tag="ssq")
            nc.vector.tensor_mul(ssq, ql, f0)
            nc.vector.scalar_tensor_tensor(ssq, qr, f1, ssq, ALU.mult, ALU.add)
            nc.vector.scalar_tensor_tensor(ssq, cr2, f2, ssq, ALU.mult, ALU.add)
            # mean_n = -mean; lets the norm fixup use (hu_x + cs*mean_n)*rinv
            # with a positive rinv (kills the eps-copy and the negate-copy;
            # eps=1e-6 is negligible vs std ~ O(1) here)
            mean_n = tinyp.tile([P, 1], f32, tag="mean")
            nc.gpsimd.tensor_scalar_mul(mean_n, sumx, -1.0 / DIM)
            m2x = tinyp.tile([P, 1], f32, tag="m2x")
            nc.vector.tensor_mul(m2x, sumx, sumx)
            varn = tinyp.tile([P, 1], f32, tag="varn")
            nc.vector.scalar_tensor_tensor(varn, m2x, -1.0 / DIM, ssq,
                                           ALU.mult, ALU.add)
            stde = tinyp.tile([P, 1], f32, tag="stde")
            nc.scalar.activation(stde, varn, ACTF.Sqrt, scale=1.0 / (DIM - 1))
            rinv = tinyp.tile([P, 1], f32, tag="rinv")
            nc.vector.reciprocal(rinv, stde)

            if stage == 5:
                stage_out(dk)
                continue

            # ---- phase D matmuls ----
            M1, M3 = unit("M1"), unit("M3")
            for c in range(4):
                lhs = _mm(hlT[:, bass.ts(c, P)], mm_dt)
                st, sp_ = (c == 0), (c == 3)
                nc.tensor.matmul(M1, lhs, _mm(wsb["hU"][:, c, :], mm_dt), start=st, stop=sp_)
                nc.tensor.matmul(M3, lhs, _mm(wsb["hU"][:, 4 + c, :], mm_dt), start=st, stop=sp_)
            M2, M4 = unit("M2"), unit("M4")
            for c in range(4):
                lhs = _mm(hrT[:, bass.ts(c, P)], mm_dt)
                st, sp_ = (c == 0), (c == 3)
                nc.tensor.matmul(M2, lhs, _mm(wsb["hU"][:, c, :], mm_dt), start=st, stop=sp_)
                nc.tensor.matmul(M4, lhs, _mm(wsb["hU"][:, 4 + c, :], mm_dt), start=st, stop=sp_)
            HSU, LSU, SBC = unit("HSU"), unit("LSU"), unit("SBC")
            for c in range(4):
                lhs = _mm(xhT[:, bass.ts(c, P)], mm_dt)
                st, sp_ = (c == 0), (c == 3)
                nc.tensor.matmul(HSU, lhs, _mm(wsb["hWu"][:, c, :], mm_dt), start=st, stop=sp_)
                nc.tensor.matmul(LSU, lhs, _mm(wsb["lWu"][:, c, :], mm_dt), start=st, stop=sp_)
                nc.tensor.matmul(SBC, lhs, _mm(wsb["hWb"][:, c, :], mm_dt), start=st, stop=sp_)
            LUp = unit("LU")
            for c in range(8):
                nc.tensor.matmul(LUp, _mm(xwT[:, bass.ts(c, P)], mm_dt),
                                 _mm(wsb["lU"][:, c, :], mm_dt),
                                 start=(c == 0), stop=(c == 7))

            # ---- hidden path: hu_x = a0*M1 + b0*M2 + a1*M3 + b1*M4 ----
            # (all on vector: the scalar queue's FIFO must stay clear so
            # next-tile transpose evictions aren't blocked behind these
            # late-dependency ops)
            t_hu = phd.tile([P, HALF], f32, tag="t_hu")
            nc.vector.tensor_scalar_mul(t_hu, M1, a0)
            nc.vector.scalar_tensor_tensor(t_hu, M2, b0, t_hu, ALU.mult, ALU.add)
            nc.vector.scalar_tensor_tensor(t_hu, M3, a1, t_hu, ALU.mult, ALU.add)
            nc.vector.scalar_tensor_tensor(t_hu, M4, b1, t_hu, ALU.mult, ALU.add)
            # t5 = rinv*(hu_x + cs*mean_n) = inv*(hu_x - cs*mean)
            t5 = phd.tile([P, HALF], f32, tag="t5")
            nc.vector.scalar_tensor_tensor(t5, bc["cs"], mean_n, t_hu,
                                           ALU.mult, ALU.add)
            nc.vector.tensor_scalar_mul(t5, t5, rinv)

            su_h = phd.tile([P, HALF], f32, tag="su_h", bufs=2)
            nc.vector.tensor_add(su_h, HSU, bc["hWu_b"])
            su_l = phd.tile([P, HALF], f32, tag="su_l", bufs=2)
            nc.vector.tensor_add(su_l, LSU, bc["lWu_b"])
            sbc = phd.tile([P, HALF], f32, tag="sbc", bufs=1)
            nc.vector.tensor_add(sbc, SBC, bc["cb"])

            v1 = phd.tile([P, HALF], f32, tag="v1")
            nc.gpsimd.tensor_mul(v1, t5, su_h)
            w1 = phd.tile([P, HALF], f32, tag="w1")
            nc.vector.tensor_mul(w1, LUp, su_l)
            tsum = phd.tile([P, HALF], f32, tag="tsum")
            nc.gpsimd.tensor_add(tsum, v1, sbc)
            out_t = outp.tile([P, HALF], f32, tag="out_t")
            nc.gpsimd.tensor_add(out_t, tsum, w1)

            nc.sync.dma_start(out_d[rs, :], out_t)

    nc.compile()
    return nc


_NC_CACHE = {}


def _get_nc(b_loc, mm_dt):
    key = (b_loc, str(mm_dt))
    if key not in _NC_CACHE:
        _NC_CACHE[key] = build_nc(b_loc, mm_dt)
    return _NC_CACHE[key]


def kernel(**inputs):
    mm_dt = f32r if os.environ.get("KERNEL_MM_DT", "f32r") == "f32r" else f32
    b = inputs["hl"].shape[0]
    n_cores = N_CORES
    b_loc = b // n_cores
    nc = _get_nc(b_loc, mm_dt)

    sharded = {"hl", "hr", "xw", "xh"}
    in_maps = []
    for i in range(n_cores):
        m = {}
        for k, v in inputs.items():
            v = np.ascontiguousarray(np.asarray(v, dtype=np.float32))
            if k in sharded:
                m[k] = v[i * b_loc:(i + 1) * b_loc]
            else:
                m[k] = v
        in_maps.append(m)

    res = run_bass_kernel_spmd(nc, in_maps, core_ids=list(range(n_cores)))
    return np.concatenate([r["out"] for r in res.results], axis=0)



# revision 21
# speedup vs baseline: 1.1498x; 1.1498x over previous
"""Trainium2 Bass kernel for nn_ChildHAggregation (gnn_message_passing).

Computation per sample:
  x = [hl, hr]                                        (1024)
  q_t = (h_t @ qU + qU_b) * su_q + sb_q   t in {l,r}  su/sb from xh
  k_t = (h_t @ kU + kU_b) * su_k + sb_k
  2x2 softmax attention over the two tokens -> per-sample probs p_ij
  x += scores @ [hl, hr]; layernorm(ddof=1) with alpha/beta
  out = (x @ hU + hU_b) * (xh @ hWu + hWu_b) + (xh @ hWb + hWb_b)
      + (xw @ lU + lU_b) * (xh @ lWu + lWu_b) + (xh @ lWb + lWb_b)

Strategy (pure data-parallel over 8 cores, batch 32768 -> 4096/core):
  - batch-major tiles [128 samples x features]; feature-contraction
    matmuls use PE-transposed input tiles as the stationary operand so
    outputs land batch-major in PSUM.  Matmul operands are bf16
    (weights cast at setup, activations cast at transpose eviction)
    so FWL accelerates the weight loads; PSUM accumulation is fp32.
  - 2-token softmax == sigmoid of score differences -> per-sample
    scalars; attention + layernorm fold into per-sample scalar algebra
    applied AFTER the matmuls (M1..M4 = h_t @ (alpha*hU) halves).
  - all "U-bias * hyper" terms folded into precomputed weights.
  - emission interleaves next-tile PE transposes BETWEEN matmul bursts
    so the PE never idles >1.2us (keeps the HAM clock-gate at 8/8),
    and spreads elementwise work across vector/scalar/gpsimd.
"""

import os
from contextlib import ExitStack

import numpy as np

import concourse.bacc as bacc
import concourse.bass as bass
import concourse.mybir as mybir
import concourse.tile as tile
from concourse.bass_utils import run_bass_kernel_spmd
from concourse.masks import make_identity

N_CORES = 8
B_FULL = 32768
HALF = 512
DIM = 1024
P = 128
EPS = 1e-6
INV_SQRT_HALF = 1.0 / float(np.sqrt(np.float32(HALF)))

f32 = mybir.dt.float32
f32r = mybir.dt.float32r
bf16 = mybir.dt.bfloat16

AX = mybir.AxisListType
ALU = mybir.AluOpType
ACTF = mybir.ActivationFunctionType

W512 = ["qU", "kU", "qWu", "qWb", "kWu", "kWb", "hWu", "hWb", "lWu", "lWb"]


def build_nc(b_loc, mm_dt=bf16):
    """Build the per-core Bass program for a local batch of b_loc rows."""
    n_tiles = b_loc // P
    assert n_tiles * P == b_loc

    # storage dtype for matmul operands (weights + transposed activations)
    st_dt = bf16 if mm_dt == bf16 else f32

    def _mm(ap):
        """View an AP in the matmul dtype (bitcast only for f32r)."""
        if mm_dt == f32r:
            return ap.bitcast(f32r)
        return ap

    nc = bacc.Bacc("TRN2", target_bir_lowering=False, debug=False,
                   num_devices=1)

    # ---- DRAM I/O (names match setup_inputs) ----
    d = {}
    d["hl"] = nc.dram_tensor("hl", [b_loc, HALF], f32, kind="ExternalInput").ap()
    d["hr"] = nc.dram_tensor("hr", [b_loc, HALF], f32, kind="ExternalInput").ap()
    d["xw"] = nc.dram_tensor("xw", [b_loc, DIM], f32, kind="ExternalInput").ap()
    d["xh"] = nc.dram_tensor("xh", [b_loc, HALF], f32, kind="ExternalInput").ap()
    for w in W512:
        d[w + "_w"] = nc.dram_tensor(w + "_w", [HALF, HALF], f32,
                                     kind="ExternalInput").ap()
        d[w + "_b"] = nc.dram_tensor(w + "_b", [HALF], f32,
                                     kind="ExternalInput").ap()
    for w in ["hU", "lU"]:
        d[w + "_w"] = nc.dram_tensor(w + "_w", [DIM, HALF], f32,
                                     kind="ExternalInput").ap()
        d[w + "_b"] = nc.dram_tensor(w + "_b", [HALF], f32,
                                     kind="ExternalInput").ap()
    d["alpha"] = nc.dram_tensor("alpha", [DIM], f32, kind="ExternalInput").ap()
    d["beta"] = nc.dram_tensor("beta", [DIM], f32, kind="ExternalInput").ap()
    out_d = nc.dram_tensor("out", [b_loc, HALF], f32, kind="ExternalOutput").ap()

    with tile.TileContext(nc) as tc, ExitStack() as ctx:
        # ================= persistent pools =================
        wts = ctx.enter_context(tc.tile_pool(name="wts", bufs=1))
        biasp = ctx.enter_context(tc.tile_pool(name="biasp", bufs=1))
        tp_ps = ctx.enter_context(tc.tile_pool(name="tp_ps", bufs=2, space="PSUM"))
        mm_ps = ctx.enter_context(tc.tile_pool(name="mm_ps", bufs=6, space="PSUM"))

        # final weight tiles, stored in the matmul dtype
        wsb = {}
        for w in ["qU", "kU", "qWu", "kWu", "hWu", "lWu", "qWb", "hWb"]:
            wsb[w] = wts.tile([P, 4, HALF], st_dt, name=f"w_{w}")
        for w in ["hU", "lU"]:
            wsb[w] = wts.tile([P, 8, HALF], st_dt, name=f"w_{w}")
        alpha_sb = wts.tile([P, 8], f32)
        nc.sync.dma_start(alpha_sb, d["alpha"].rearrange("(c p) -> p c", p=P))
        beta_sb = wts.tile([P, 8], f32)
        nc.sync.dma_start(beta_sb, d["beta"].rearrange("(c p) -> p c", p=P))
        ident = wts.tile([P, P], f32)
        make_identity(nc, ident)

        # persistent broadcast [P, 512] bias tiles
        bc = {}
        for nm in ["qWu_b", "kWu_b", "hWu_b", "lWu_b", "qb", "cb", "cs"]:
            bc[nm] = biasp.tile([P, HALF], f32, name=f"bc_{nm}")

        # ================= input pool + prefetch =================
        inp = ctx.enter_context(tc.tile_pool(name="inp", bufs=3))
        tsp = ctx.enter_context(tc.tile_pool(name="tsp", bufs=2))
        prefetched = {}

        def fetch_inputs(i):
            for nm, wdt in (("hl", HALF), ("hr", HALF), ("xh", HALF),
                            ("xw", DIM)):
                t = inp.tile([P, wdt], f32, tag=nm, name=f"in_{nm}_{i}")
                nc.sync.dma_start(t, d[nm][bass.ts(i, P), :])
                prefetched[(i, nm)] = t

        for i in range(min(2, n_tiles)):
            fetch_inputs(i)

        prehd = {}

        def make_hd(i):
            # hd = hl - hr on gpsimd (SBUF-only engine with slack)
            t = inp.tile([P, HALF], f32, tag="hd", name=f"hd_{i}")
            nc.gpsimd.tensor_sub(t, prefetched[(i, "hl")],
                                 prefetched[(i, "hr")])
            prehd[i] = t

        make_hd(0)

        # ---- transpose helpers ----
        preT = {}

        def tgroup(src, c0, gw, tg, i):
            """Transpose gw 128-col chunks [c0:c0+gw] of src into preT tile."""
            key = (i, tg)
            if key not in preT:
                ncols = 8 if tg == "TxW" else 4
                preT[key] = tsp.tile([P, ncols * P], st_dt, tag=tg,
                                     name=f"T_{tg}_{i}")
            sb = preT[key]
            ps = tp_ps.tile([P, 4 * P], f32, tag="tp", name=f"tps_{tg}_{c0}_{i}")
            for c in range(gw):
                nc.tensor.transpose(ps[:, c * P:(c + 1) * P],
                                    src[:, (c0 + c) * P:(c0 + c + 1) * P],
                                    ident)
            # eviction casts to the matmul storage dtype
            nc.scalar.copy(_mm(sb[:, c0 * P:(c0 + gw) * P]), ps[:, :gw * P])
            return sb

        SRC_OF = {"ThL": "hl", "ThR": "hr", "ThD": "hd", "TxH": "xh",
                  "TxW": "xw"}

        def tpose(i, tg, c0=0, gw=4):
            nm = SRC_OF[tg]
            src = prehd[i] if nm == "hd" else prefetched[(i, nm)]
            return tgroup(src, c0, gw, tg, i)

        # pre-loop: transpose all of tile 0 (PE busy while weight DMAs land)
        for tg in ["ThL", "ThR", "ThD", "TxH"]:
            tpose(0, tg)
        tpose(0, "TxW", 0, 4)
        tpose(0, "TxW", 4, 4)

        # ---------------- one-time setup ----------------
        with tc.tile_pool(name="setup", bufs=1) as sp:
            _rows = {}

            def bias_row(nm, dedicated=False):
                if nm not in _rows:
                    tag = f"row_{nm}" if dedicated else "row"
                    r = sp.tile([1, HALF], f32, tag=tag,
                                bufs=(1 if dedicated else 2), name=f"row_{nm}")
                    nc.sync.dma_start(r, d[nm][None, :])
                    _rows[nm] = r
                return _rows[nm]

            # late-consumed rows: dedicated buffers, DMAs issued up-front
            for nm in ["hU_b", "hWb_b", "lWb_b"]:
                bias_row(nm, dedicated=True)

            def bcast(dst, row_ap):
                nc.gpsimd.partition_broadcast(dst, row_ap)

            def tmp_bc(nm, row_ap):
                t = sp.tile([P, HALF], f32, tag="tbc", bufs=4, name=f"tbc_{nm}")
                bcast(t, row_ap)
                return t

            for nm in ["qWu_b", "kWu_b", "hWu_b", "lWu_b"]:
                bcast(bc[nm], bias_row(nm))

            qUb_bc = tmp_bc("qU_b", bias_row("qU_b"))
            lUb_bc = tmp_bc("lU_b", bias_row("lU_b"))

            # qb' = qWb_b + qU_b*qWu_b
            ftmp = sp.tile([P, HALF], f32, tag="ftmp")
            bcast(bc["qb"], bias_row("qWb_b"))
            nc.vector.tensor_mul(ftmp, qUb_bc, bc["qWu_b"])
            nc.vector.tensor_add(bc["qb"], bc["qb"], ftmp)

            # weight temps: [P,4,HALF] chunks on one rotating tag
            def wtemp(w, c0=0):
                t = sp.tile([P, 4, HALF], f32, tag="wtmp4", bufs=3,
                            name=f"wtmp_{w}_{c0}")
                rr = d[w + "_w"].rearrange("(c p) o -> p c o", p=P)
                for c in range(4):
                    nc.sync.dma_start(t[:, c, :], rr[:, c0 + c, :])
                return t

            # hU first: its setup matmuls head the PE queue after the
            # pre-loop transposes
            hU_tmps = [wtemp("hU", 0), wtemp("hU", 4)]
            bh_full = mm_ps.tile([P, HALF], f32, tag="mm", name="ps_bh_setup")
            cs_full = mm_ps.tile([P, HALF], f32, tag="mm", name="ps_cs_setup")
            bh_ps = bh_full[0:1, :]
            cs_ps = cs_full[0:1, :]
            for c in range(8):
                nc.tensor.matmul(bh_ps, beta_sb[:, c:c + 1],
                                 hU_tmps[c // 4][:, c % 4, :],
                                 start=(c == 0), stop=(c == 7))
            for c in range(8):
                nc.tensor.matmul(cs_ps, alpha_sb[:, c:c + 1],
                                 hU_tmps[c // 4][:, c % 4, :],
                                 start=(c == 0), stop=(c == 7))
            bh_row = sp.tile([1, HALF], f32, tag="row", bufs=2)
            nc.vector.tensor_add(bh_row, bh_ps, bias_row("hU_b"))
            cs_row = sp.tile([1, HALF], f32, tag="row", bufs=2)
            nc.vector.tensor_copy(cs_row, cs_ps)
            bcast(bc["cs"], cs_row)
            bh_bc = tmp_bc("bh", bh_row)
            for c in range(8):
                nc.vector.tensor_scalar_mul(_mm(wsb["hU"][:, c, :]),
                                            hU_tmps[c // 4][:, c % 4, :],
                                            alpha_sb[:, c:c + 1])

            wt_tmps = {}
            for w in ["qU", "kU", "qWu", "kWu"]:
                t = wt_tmps[w] = wtemp(w)
                for c in range(4):
                    nc.vector.tensor_copy(_mm(wsb[w][:, c, :]), t[:, c, :])
            for w in ["hWu", "lWu"]:
                t = wt_tmps[w] = wtemp(w)
                for c in range(4):
                    nc.vector.tensor_copy(_mm(wsb[w][:, c, :]), t[:, c, :])

            for half in range(2):
                lU_tmp = wtemp("lU", 4 * half)
                for c in range(4):
                    nc.vector.tensor_copy(_mm(wsb["lU"][:, 4 * half + c, :]),
                                          lU_tmp[:, c, :])

            # cb = (hWb_b + bh*hWu_b) + (lWb_b + lU_b*lWu_b)
            bcast(bc["cb"], bias_row("hWb_b"))
            nc.vector.tensor_mul(ftmp, bh_bc, bc["hWu_b"])
            nc.vector.tensor_add(bc["cb"], bc["cb"], ftmp)
            lWbb_bc = tmp_bc("lWb_b", bias_row("lWb_b"))
            nc.vector.tensor_add(bc["cb"], bc["cb"], lWbb_bc)
            nc.vector.tensor_mul(ftmp, lUb_bc, bc["lWu_b"])
            nc.vector.tensor_add(bc["cb"], bc["cb"], ftmp)

            # folded weights: qWb' = qWb + qWu*diag(qU_b)
            # WC = hWb + hWu*diag(bh) + lWb + lWu*diag(lU_b)
            qWb_tmp = wtemp("qWb")
            for c in range(4):
                nc.vector.tensor_mul(ftmp, _mm(wsb["qWu"][:, c, :]), qUb_bc)
                nc.vector.tensor_add(_mm(wsb["qWb"][:, c, :]),
                                     qWb_tmp[:, c, :], ftmp)
            hWb_tmp = wtemp("hWb")
            lWb_tmp = wtemp("lWb")
            for c in range(4):
                nc.vector.tensor_mul(ftmp, _mm(wsb["hWu"][:, c, :]), bh_bc)
                nc.vector.tensor_add(hWb_tmp[:, c, :], hWb_tmp[:, c, :], ftmp)
                nc.vector.tensor_add(hWb_tmp[:, c, :], hWb_tmp[:, c, :],
                                     lWb_tmp[:, c, :])
                nc.vector.tensor_mul(ftmp, _mm(wsb["lWu"][:, c, :]), lUb_bc)
                nc.vector.tensor_add(_mm(wsb["hWb"][:, c, :]),
                                     hWb_tmp[:, c, :], ftmp)

        # ================= main loop pools =================
        pha = ctx.enter_context(tc.tile_pool(name="pha", bufs=1))
        scr = ctx.enter_context(tc.tile_pool(name="scr", bufs=3))
        tinyp = ctx.enter_context(tc.tile_pool(name="tinyp", bufs=2))
        phd = ctx.enter_context(tc.tile_pool(name="phd", bufs=1))
        outp = ctx.enter_context(tc.tile_pool(name="outp", bufs=2))

        smask = int(os.environ.get("KERNEL_STATS_MASK", "7"))

        for i in range(n_tiles):
            rs = bass.ts(i, P)
            # ---- pipeline: fetch tile i+2 inputs, hd for tile i+1 ----
            if i + 2 < n_tiles:
                fetch_inputs(i + 2)
            if i + 1 < n_tiles:
                make_hd(i + 1)

            hl_t = prefetched.pop((i, "hl"))
            hr_t = prefetched.pop((i, "hr"))
            xh_t = prefetched.pop((i, "xh"))
            xw_t = prefetched.pop((i, "xw"))
            hd_t = prehd.pop(i)
            hlT = preT.pop((i, "ThL"))
            hrT = preT.pop((i, "ThR"))
            hdT = preT.pop((i, "ThD"))
            xhT = preT.pop((i, "TxH"))
            xwT = preT.pop((i, "TxW"))

            # ---- row stats of hl / hr (ready at iteration start) ----
            sl = tinyp.tile([P, 1], f32, tag="sl")
            sr = tinyp.tile([P, 1], f32, tag="sr")
            ql = tinyp.tile([P, 1], f32, tag="ql")
            qr = tinyp.tile([P, 1], f32, tag="qr")
            cr2 = tinyp.tile([P, 1], f32, tag="cr2")
            if smask & 1:
                s4 = scr.tile([P, HALF], f32, tag="scr", name=f"scr_sl_{i}")
                nc.scalar.activation(s4, hl_t, ACTF.Copy, accum_out=sl)
                s5 = scr.tile([P, HALF], f32, tag="scr", name=f"scr_sr_{i}")
                nc.scalar.activation(s5, hr_t, ACTF.Copy, accum_out=sr)
            if smask & 2:
                s1 = scr.tile([P, HALF], f32, tag="scr", name=f"scr_ql_{i}")
                nc.vector.scalar_tensor_tensor(
                    s1, hl_t, 0.0, hl_t, ALU.bypass, ALU.mult, accum_out=ql)
                s2 = scr.tile([P, HALF], f32, tag="scr", name=f"scr_qr_{i}")
                nc.vector.scalar_tensor_tensor(
                    s2, hr_t, 0.0, hr_t, ALU.bypass, ALU.mult, accum_out=qr)
            if smask & 4:
                # 2*hl*hr (the 2 folds the old f2 doubling)
                s3 = scr.tile([P, HALF], f32, tag="scr", name=f"scr_cr_{i}")
                nc.vector.scalar_tensor_tensor(
                    s3, hl_t, 2.0, hr_t, ALU.mult, ALU.mult, accum_out=cr2)

            def unit(tag):
                return mm_ps.tile([P, HALF], f32, tag="mm", name=f"ps_{tag}_{i}")

            # ---- PE burst 1: xh-block (SUq/SBq/TU share each xhT chunk) ----
            SUq, SBq, TU = unit("SUq"), unit("SBq"), unit("TU")
            for c in range(4):
                lhs = _mm(xhT[:, bass.ts(c, P)])
                st, sp_ = (c == 0), (c == 3)
                nc.tensor.matmul(SUq, lhs, _mm(wsb["qWu"][:, c, :]), start=st, stop=sp_)
                nc.tensor.matmul(SBq, lhs, _mm(wsb["qWb"][:, c, :]), start=st, stop=sp_)
                nc.tensor.matmul(TU, lhs, _mm(wsb["kWu"][:, c, :]), start=st, stop=sp_)

            # ---- transposes for tile i+1 (group 1) ----
            if i + 1 < n_tiles:
                tpose(i + 1, "ThL")
                tpose(i + 1, "ThR")

            # ---- vector: phase-A hyper terms ----
            su = pha.tile([P, HALF], f32, tag="su", bufs=2)
            nc.vector.tensor_add(su, SUq, bc["qWu_b"])
            sbq = pha.tile([P, HALF], f32, tag="sbq", bufs=2)
            nc.vector.tensor_add(sbq, SBq, bc["qb"])
            tu = pha.tile([P, HALF], f32, tag="tu", bufs=2)
            nc.vector.tensor_add(tu, TU, bc["kWu_b"])

            # ---- PE burst 2: CD, then hl-block (A_l/M1/M3 share hlT) ----
            CD = unit("CD")
            for c in range(4):
                nc.tensor.matmul(CD, _mm(hdT[:, bass.ts(c, P)]),
                                 _mm(wsb["kU"][:, c, :]),
                                 start=(c == 0), stop=(c == 3))
            A_l, M1, M3 = unit("A_l"), unit("M1"), unit("M3")
            for c in range(4):
                lhs = _mm(hlT[:, bass.ts(c, P)])
                st, sp_ = (c == 0), (c == 3)
                nc.tensor.matmul(A_l, lhs, _mm(wsb["qU"][:, c, :]), start=st, stop=sp_)
                nc.tensor.matmul(M1, lhs, _mm(wsb["hU"][:, c, :]), start=st, stop=sp_)
                nc.tensor.matmul(M3, lhs, _mm(wsb["hU"][:, 4 + c, :]), start=st, stop=sp_)

            # ---- transposes for tile i+1 (group 2) ----
            if i + 1 < n_tiles:
                tpose(i + 1, "ThD")
                tpose(i + 1, "TxH")

            # ---- vector: dk = (k_l - k_r) factor, u = su*dk ----
            dk = pha.tile([P, HALF], f32, tag="dk")
            nc.vector.tensor_mul(dk, CD, tu)
            u = pha.tile([P, HALF], f32, tag="u")
            nc.vector.tensor_mul(u, su, dk)

            # ---- PE burst 3: hr-block (A_r/M2/M4 share hrT) ----
            A_r, M2, M4 = unit("A_r"), unit("M2"), unit("M4")
            for c in range(4):
                lhs = _mm(hrT[:, bass.ts(c, P)])
                st, sp_ = (c == 0), (c == 3)
                nc.tensor.matmul(A_r, lhs, _mm(wsb["qU"][:, c, :]), start=st, stop=sp_)
                nc.tensor.matmul(M2, lhs, _mm(wsb["hU"][:, c, :]), start=st, stop=sp_)
                nc.tensor.matmul(M4, lhs, _mm(wsb["hU"][:, 4 + c, :]), start=st, stop=sp_)

            # ---- transposes for tile i+1 (group 3) ----
            if i + 1 < n_tiles:
                tpose(i + 1, "TxW", 0, 4)
                tpose(i + 1, "TxW", 4, 4)

            # ---- dots -> 2-way softmax scalars ----
            stats = tinyp.tile([P, 4], f32, tag="stats")
            for j, (aa, bb) in enumerate([(sbq, dk), (A_l, u), (A_r, u)]):
                sd = scr.tile([P, HALF], f32, tag="scr", name=f"scr_dot{j}_{i}")
                nc.vector.scalar_tensor_tensor(
                    sd, aa, 0.0, bb, ALU.bypass, ALU.mult,
                    accum_out=stats[:, j:j + 1])
            diffs = tinyp.tile([P, 2], f32, tag="diffs")
            nc.vector.tensor_add(diffs, stats[:, 1:3],
                                 stats[:, 0:1].broadcast_to([P, 2]))
            probs = tinyp.tile([P, 2], f32, tag="probs")
            nc.scalar.activation(probs, diffs, ACTF.Sigmoid, scale=INV_SQRT_HALF)
            a0 = tinyp.tile([P, 1], f32, tag="a0")
            nc.vector.tensor_scalar_add(a0, probs[:, 0:1], 1.0)
            b0 = tinyp.tile([P, 1], f32, tag="b0")
            nc.vector.tensor_scalar(b0, probs[:, 0:1], -1.0, 1.0,
                                    ALU.mult, ALU.add)
            a1 = probs[:, 1:2]
            b1 = tinyp.tile([P, 1], f32, tag="b1")
            nc.vector.tensor_scalar(b1, probs[:, 1:2], -1.0, 2.0,
                                    ALU.mult, ALU.add)

            # ---- hidden combine hu_x = a0*M1 + b0*M2 + a1*M3 + b1*M4 ----
            # (early: frees the M1-4 PSUM banks before burst 4 reuses them)
            t_hu = phd.tile([P, HALF], f32, tag="t_hu")
            nc.scalar.activation(t_hu, M1, ACTF.Copy, scale=a0)
            nc.vector.scalar_tensor_tensor(t_hu, M2, b0, t_hu, ALU.mult, ALU.add)
            nc.vector.scalar_tensor_tensor(t_hu, M3, a1, t_hu, ALU.mult, ALU.add)
            nc.vector.scalar_tensor_tensor(t_hu, M4, b1, t_hu, ALU.mult, ALU.add)

            # ---- PE burst 4: xh phase-D block + LUp ----
            HSU, LSU, SBC = unit("HSU"), unit("LSU"), unit("SBC")
            for c in range(4):
                lhs = _mm(xhT[:, bass.ts(c, P)])
                st, sp_ = (c == 0), (c == 3)
                nc.tensor.matmul(HSU, lhs, _mm(wsb["hWu"][:, c, :]), start=st, stop=sp_)
                nc.tensor.matmul(LSU, lhs, _mm(wsb["lWu"][:, c, :]), start=st, stop=sp_)
                nc.tensor.matmul(SBC, lhs, _mm(wsb["hWb"][:, c, :]), start=st, stop=sp_)
            LUp = unit("LU")
            for c in range(8):
                nc.tensor.matmul(LUp, _mm(xwT[:, bass.ts(c, P)]),
                                 _mm(wsb["lU"][:, c, :]),
                                 start=(c == 0), stop=(c == 7))

            # ---- layernorm stats from folded algebra ([P,1] ops, vector) ----
            e0 = tinyp.tile([P, 1], f32, tag="e0")
            nc.vector.tensor_add(e0, a0, a1)
            e1 = tinyp.tile([P, 1], f32, tag="e1")
            nc.vector.tensor_add(e1, b0, b1)
            sumx = tinyp.tile([P, 1], f32, tag="sumx")
            nc.vector.tensor_mul(sumx, sl, e0)
            nc.vector.scalar_tensor_tensor(sumx, sr, e1, sumx, ALU.mult, ALU.add)
            f0 = tinyp.tile([P, 1], f32, tag="f0")
            nc.vector.tensor_mul(f0, a0, a0)
            nc.vector.scalar_tensor_tensor(f0, a1, a1, f0, ALU.mult, ALU.add)
            f1 = tinyp.tile([P, 1], f32, tag="f1")
            nc.vector.tensor_mul(f1, b0, b0)
            nc.vector.scalar_tensor_tensor(f1, b1, b1, f1, ALU.mult, ALU.add)
            f2 = tinyp.tile([P, 1], f32, tag="f2")
            nc.vector.tensor_mul(f2, a0, b0)
            nc.vector.scalar_tensor_tensor(f2, a1, b1, f2, ALU.mult, ALU.add)
            ssq = tinyp.tile([P, 1], f32, tag="ssq")
            nc.vector.tensor_mul(ssq, ql, f0)
            nc.vector.scalar_tensor_tensor(ssq, qr, f1, ssq, ALU.mult, ALU.add)
            nc.vector.scalar_tensor_tensor(ssq, cr2, f2, ssq, ALU.mult, ALU.add)
            mean_n = tinyp.tile([P, 1], f32, tag="mean")
            nc.vector.tensor_scalar_mul(mean_n, sumx, -1.0 / DIM)
            m2x = tinyp.tile([P, 1], f32, tag="m2x")
            nc.vector.tensor_mul(m2x, sumx, sumx)
            varn = tinyp.tile([P, 1], f32, tag="varn")
            nc.vector.scalar_tensor_tensor(varn, m2x, -1.0 / DIM, ssq,
                                           ALU.mult, ALU.add)
            stde = tinyp.tile([P, 1], f32, tag="stde")
            nc.scalar.activation(stde, varn, ACTF.Sqrt, scale=1.0 / (DIM - 1))
            rinv = tinyp.tile([P, 1], f32, tag="rinv")
            nc.vector.reciprocal(rinv, stde)

            # ---- norm fixup: t5 = rinv*(hu_x + cs*mean_n) ----
            t5 = phd.tile([P, HALF], f32, tag="t5")
            nc.vector.scalar_tensor_tensor(t5, bc["cs"], mean_n, t_hu,
                                           ALU.mult, ALU.add)
            nc.scalar.activation(t5, t5, ACTF.Copy, scale=rinv)

            # ---- phase D combine ----
            su_h = phd.tile([P, HALF], f32, tag="su_h", bufs=2)
            nc.vector.tensor_add(su_h, HSU, bc["hWu_b"])
            su_l = phd.tile([P, HALF], f32, tag="su_l", bufs=2)
            nc.vector.tensor_add(su_l, LSU, bc["lWu_b"])
            sbc = phd.tile([P, HALF], f32, tag="sbc", bufs=1)
            nc.vector.tensor_add(sbc, SBC, bc["cb"])

            v1 = phd.tile([P, HALF], f32, tag="v1")
            nc.gpsimd.tensor_mul(v1, t5, su_h)
            w1 = phd.tile([P, HALF], f32, tag="w1")
            nc.vector.tensor_mul(w1, LUp, su_l)
            tsum = phd.tile([P, HALF], f32, tag="tsum")
            nc.gpsimd.tensor_add(tsum, v1, sbc)
            out_t = outp.tile([P, HALF], f32, tag="out_t")
            nc.gpsimd.tensor_add(out_t, tsum, w1)

            nc.sync.dma_start(out_d[rs, :], out_t)

    nc.compile()
    return nc


_NC_CACHE = {}


def _get_nc(b_loc, mm_dt):
    key = (b_loc, str(mm_dt))
    if key not in _NC_CACHE:
        _NC_CACHE[key] = build_nc(b_loc, mm_dt)
    return _NC_CACHE[key]


def _env_mm_dt():
    s = os.environ.get("KERNEL_MM_DT", "bf16")
    return {"bf16": bf16, "f32r": f32r, "f32": f32}[s]


def kernel(**inputs):
    mm_dt = _env_mm_dt()
    b = inputs["hl"].shape[0]
    n_cores = N_CORES
    b_loc = b // n_cores
    nc = _get_nc(b_loc, mm_dt)

    sharded = {"hl", "hr", "xw", "xh"}
    in_maps = []
    for i in range(n_cores):
        m = {}
        for k, v in inputs.items():
            v = np.ascontiguousarray(np.asarray(v, dtype=np.float32))
            if k in sharded:
                m[k] = v[i * b_loc:(i + 1) * b_loc]
            else:
                m[k] = v
        in_maps.append(m)

    res = run_bass_kernel_spmd(nc, in_maps, core_ids=list(range(n_cores)))
    return np.concatenate([r["out"] for r in res.results], axis=0)
